# revision 6
# baseline (speedup 1.0000x reference)
"""Trainium2 Bass kernel for nn_DSI_beta (3-hop DSI pooling).

Reference computation (per batch b):
    s = state_emb[b]           # [S=512, D=512]
    q = query_emb[b]           # [D=512]
    for i in 0..2:
        dist = softmax(10 * cos(q, s_rows))            # [S]
        s = leaky_relu(s @ W[i].T + b[i], 0.01)        # [S, D]
        q = LN(q + dist @ s)                           # [D]
    final_dist = softmax(10 * cos(q, s_rows))
    returns (q, final_dist)

Sharding: data-parallel over batch, 32 batches per NeuronCore (8 cores).

On-chip layout (per batch):
  s is kept TRANSPOSED: sT[t] = s.T[t*128:(t+1)*128, :]  -> 4 tiles [128d, 512S].
  The hop matmul out[e, j] = sum_d W'[d,e] * sT[d,j] keeps the layout invariant
  (weight-stationary on TensorE, fp32r = TF32-like precision, fp32 accumulate).
  q lives as a column tile [128, 4] (q[t*128+p] = qcol[p, t]).
  dot/sumsq rows come from TensorE matvecs; softmax runs on [1,512] rows;
  the weighted aggregation uses a broadcast of exp(logits) over partitions
  (rank-1 matmul) followed by fused multiply+free-reduce on VectorE.
"""

import sys

if "/opt/trn_rl_repo" not in sys.path:
    sys.path.insert(0, "/opt/trn_rl_repo")

from contextlib import ExitStack

import numpy as np

import concourse.tile as tile
from concourse import bacc, bass_isa, mybir
from concourse.bass_utils import run_bass_kernel_spmd

F32 = mybir.dt.float32
F32R = mybir.dt.float32r
AX = mybir.AxisListType
OP = mybir.AluOpType
AF = mybir.ActivationFunctionType

N_CORES = 8
B, S, D = 256, 512, 512
BL = B // N_CORES  # 32 batches per core
NT = D // 128      # 4 partition tiles
N_HOP = 3
TEMP = 10.0
SLOPE = 0.01
EPS_LN = 1e-5

_cached = {}


def _round_f32r(x: np.ndarray) -> np.ndarray:
    """Round fp32 to the 11-bit-mantissa fp32r format (bytes stay fp32)."""
    u = np.ascontiguousarray(x, dtype=np.float32).view(np.uint32)
    u = (u + np.uint32(0x800)) & np.uint32(0xFFFFF000)
    return u.view(np.float32)


def _build(rep=1):
    nc = bacc.Bacc("TRN2", target_bir_lowering=False, debug=False,
                   num_devices=N_CORES)

    state_T = nc.dram_tensor("state_T", [BL, D, S], F32R, kind="ExternalInput").ap()
    q_cols = nc.dram_tensor("q_cols", [BL, 128, NT], F32R, kind="ExternalInput").ap()
    Wt = nc.dram_tensor("Wt", [N_HOP, D, D], F32R, kind="ExternalInput").ap()
    b_cols = nc.dram_tensor("b_cols", [N_HOP, 128, NT], F32, kind="ExternalInput").ap()
    g_col = nc.dram_tensor("g_col", [128, NT], F32, kind="ExternalInput").ap()
    be_col = nc.dram_tensor("be_col", [128, NT], F32, kind="ExternalInput").ap()
    ones_c = nc.dram_tensor("ones_c", [128, 1], F32R, kind="ExternalInput").ap()
    ones_r = nc.dram_tensor("ones_r", [1, 128], F32R, kind="ExternalInput").ap()
    eye_in = nc.dram_tensor("eye_in", [128, 128], F32, kind="ExternalInput").ap()

    q_out = nc.dram_tensor("q_out", [BL, D], F32, kind="ExternalOutput").ap()
    d_out = nc.dram_tensor("d_out", [BL, S], F32, kind="ExternalOutput").ap()

    with tile.TileContext(nc) as tc, ExitStack() as ctx:
        const = ctx.enter_context(tc.tile_pool(name="const", bufs=1))
        sbST = ctx.enter_context(tc.tile_pool(name="sbST", bufs=12))
        sbSQ = ctx.enter_context(tc.tile_pool(name="sbSQ", bufs=12))
        sbQ = ctx.enter_context(tc.tile_pool(name="sbQ", bufs=4))
        sbRow = ctx.enter_context(tc.tile_pool(name="sbRow", bufs=4))
        sbSm = ctx.enter_context(tc.tile_pool(name="sbSm", bufs=6))
        sbJk = ctx.enter_context(tc.tile_pool(name="sbJk", bufs=3))
        psMM = ctx.enter_context(tc.tile_pool(name="psMM", bufs=2, space="PSUM"))
        psDot = ctx.enter_context(tc.tile_pool(name="psDot", bufs=1, space="PSUM"))
        psSS = ctx.enter_context(tc.tile_pool(name="psSS", bufs=1, space="PSUM"))
        psE = ctx.enter_context(tc.tile_pool(name="psE", bufs=1, space="PSUM"))
        psQ = ctx.enter_context(tc.tile_pool(name="psQ", bufs=1, space="PSUM"))

        # ---- constants (loaded once) ----
        Wtiles = []  # Wtiles[i][t]: [128, 512] = Wt[i, t*128:(t+1)*128, :]
        for i in range(N_HOP):
            row = []
            for t in range(NT):
                w = const.tile([128, D], F32R, name=f"W_{i}_{t}")
                nc.sync.dma_start(w[:], Wt[i, t * 128:(t + 1) * 128, :])
                row.append(w)
            Wtiles.append(row)
        bcols = []
        for i in range(N_HOP):
            bc = const.tile([128, NT], F32, name=f"b_{i}")
            nc.sync.dma_start(bc[:], b_cols[i])
            bcols.append(bc)
        gcol = const.tile([128, NT], F32, name="gcol")
        nc.sync.dma_start(gcol[:], g_col)
        becol = const.tile([128, NT], F32, name="becol")
        nc.sync.dma_start(becol[:], be_col)
        onesc = const.tile([128, 1], F32R, name="onesc")
        nc.sync.dma_start(onesc[:], ones_c)
        onesr = const.tile([1, 128], F32R, name="onesr")
        nc.sync.dma_start(onesr[:], ones_r)
        eye = const.tile([128, 128], F32, name="eye")
        nc.sync.dma_start(eye[:], eye_in)
        epsln = const.tile([128, 1], F32, name="epsln")
        nc.vector.memset(epsln[:], EPS_LN)

        def allred(dst, src):
            nc.gpsimd.partition_all_reduce(dst[:], src[:], channels=128,
                                           reduce_op=bass_isa.ReduceOp.add)

        for _rep in range(rep):
          for b in range(BL):
            # ---- load state (transposed on host) and query column ----
            sT = []
            for t in range(NT):
                st = sbST.tile([128, S], F32R, name="sT")
                nc.sync.dma_start(st[:], state_T[b, t * 128:(t + 1) * 128, :])
                sT.append(st)
            qcol = sbQ.tile([128, NT], F32R, name="qcol")
            nc.sync.dma_start(qcol[:], q_cols[b])

            # squares of s (for row norms)
            sq = []
            for t in range(NT):
                s2 = sbSQ.tile([128, S], F32R, name="sq")
                nc.scalar.activation(s2[:], sT[t][:].bitcast(F32), AF.Square)
                sq.append(s2)

            # ||q||^2 replicated across partitions
            junk4 = sbSm.tile([128, NT], F32, name="junk4")
            qn2p = sbSm.tile([128, 1], F32, name="qn2p")
            nc.vector.scalar_tensor_tensor(
                out=junk4[:], in0=qcol[:].bitcast(F32), scalar=1.0,
                in1=qcol[:].bitcast(F32), op0=OP.mult, op1=OP.mult,
                accum_out=qn2p[:])
            qn2r = sbSm.tile([128, 1], F32, name="qn2r")
            allred(qn2r, qn2p)

            def cos_softmax_rows(sT_i, sq_i, qcol_i, qn2r_i, erow_dtype):
                """dot/sumsq matvecs + row softmax numerator.

                Returns (erow [1,512] tile of erow_dtype, esum [1,1] f32).
                erow = exp(10 * cos), esum = sum(erow).
                """
                dotp = psDot.tile([1, S], F32, name="dotp")
                for t in range(NT):
                    nc.tensor.matmul(dotp[0:1, :], qcol_i[:, t:t + 1], sT_i[t][:],
                                     start=(t == 0), stop=(t == NT - 1))
                ssp = psSS.tile([1, S], F32, name="ssp")
                for t in range(NT):
                    nc.tensor.matmul(ssp[0:1, :], onesc[:], sq_i[t][:],
                                     start=(t == 0), stop=(t == NT - 1))
                # sn*qn = sqrt(sumsq * qn^2); rec = 1/(sn*qn); logits*10 in exp
                snqn = sbRow.tile([1, S], F32, name="snqn")
                nc.scalar.activation(snqn[:], ssp[0:1, :], AF.Sqrt,
                                     scale=qn2r_i[0:1, 0:1])
                rec = sbRow.tile([1, S], F32, name="rec")
                nc.vector.reciprocal(rec[:], snqn[:])
                lg = sbRow.tile([1, S], F32, name="lg")
                nc.vector.tensor_mul(lg[:], dotp[0:1, :], rec[:])
                erow = sbRow.tile([1, S], erow_dtype, name="erow")
                esum = sbSm.tile([1, 1], F32, name="esum")
                nc.scalar.activation(erow[:], lg[:], AF.Exp, scale=TEMP,
                                     accum_out=esum[:])
                return erow, esum

            for i in range(N_HOP):
                erow, esum = cos_softmax_rows(sT, sq, qcol, qn2r, F32R)

                # ---- big matmul: s_{i+1}^T[e, j] = lrelu(sum_d W'[d,e] sT[d,j] + b[e])
                sT1, sq1 = [], []
                for m in range(NT):
                    mmp = psMM.tile([128, S], F32, name="mmp")
                    for t in range(NT):
                        nc.tensor.matmul(
                            mmp[:], Wtiles[i][t][:, m * 128:(m + 1) * 128],
                            sT[t][:], start=(t == 0), stop=(t == NT - 1))
                    st1 = sbST.tile([128, S], F32R, name="sT")
                    nc.scalar.activation(st1[:], mmp[:], AF.Lrelu,
                                         bias=bcols[i][:, m:m + 1], scale=1.0,
                                         alpha=SLOPE)
                    sT1.append(st1)
                    s21 = sbSQ.tile([128, S], F32R, name="sq")
                    nc.scalar.activation(s21[:], st1[:].bitcast(F32), AF.Square)
                    sq1.append(s21)

                # ---- agg[e] = sum_j erow[j] * s_{i+1}^T[e, j] ----
                Ep = psE.tile([128, S], F32, name="Ep")
                nc.tensor.matmul(Ep[:], onesr[:], erow[:], start=True, stop=True)
                aggc = sbSm.tile([128, NT], F32, name="aggc")
                for t in range(NT):
                    jk = sbJk.tile([128, S], F32, name="jk")
                    nc.vector.scalar_tensor_tensor(
                        out=jk[:], in0=sT1[t][:].bitcast(F32), scalar=1.0,
                        in1=Ep[:], op0=OP.mult, op1=OP.mult,
                        accum_out=aggc[:, t:t + 1])

                # ---- q = LN(q + agg/esum) in column land ----
                recS = sbSm.tile([1, 1], F32, name="recS")
                nc.vector.reciprocal(recS[:], esum[:])
                recSb = sbSm.tile([128, 1], F32, name="recSb")
                nc.gpsimd.partition_broadcast(recSb[:], recS[:], channels=128)
                qpre = sbSm.tile([128, NT], F32, name="qpre")
                qsum = sbSm.tile([128, 1], F32, name="qsum")
                nc.vector.scalar_tensor_tensor(
                    out=qpre[:], in0=aggc[:], scalar=recSb[:],
                    in1=qcol[:].bitcast(F32), op0=OP.mult, op1=OP.add,
                    accum_out=qsum[:])
                qsumr = sbSm.tile([128, 1], F32, name="qsumr")
                allred(qsumr, qsum)
                negmu = sbSm.tile([128, 1], F32, name="negmu")
                nc.vector.tensor_scalar(out=negmu[:], in0=qsumr[:],
                                        scalar1=-1.0 / D, scalar2=None,
                                        op0=OP.mult)
                center = sbSm.tile([128, NT], F32, name="center")
                nc.vector.tensor_scalar(out=center[:], in0=qpre[:],
                                        scalar1=negmu[:], scalar2=None,
                                        op0=OP.add)
                junk4b = sbSm.tile([128, NT], F32, name="junk4b")
                vs = sbSm.tile([128, 1], F32, name="vs")
                nc.vector.scalar_tensor_tensor(
                    out=junk4b[:], in0=center[:], scalar=1.0, in1=center[:],
                    op0=OP.mult, op1=OP.mult, accum_out=vs[:])
                vsr = sbSm.tile([128, 1], F32, name="vsr")
                allred(vsr, vs)
                stdv = sbSm.tile([128, 1], F32, name="stdv")
                nc.scalar.activation(stdv[:], vsr[:], AF.Sqrt, scale=1.0 / D,
                                     bias=epsln[:, 0:1])
                rstd = sbSm.tile([128, 1], F32, name="rstd")
                nc.vector.reciprocal(rstd[:], stdv[:])
                qtmp = sbSm.tile([128, NT], F32, name="qtmp")
                nc.vector.scalar_tensor_tensor(
                    out=qtmp[:], in0=center[:], scalar=rstd[:], in1=gcol[:],
                    op0=OP.mult, op1=OP.mult)
                qcol = sbQ.tile([128, NT], F32R, name="qcol")
                nc.vector.tensor_tensor(out=qcol[:], in0=qtmp[:], in1=becol[:],
                                        op=OP.add)
                # ||q||^2 for the next cosine
                junk4c = sbSm.tile([128, NT], F32, name="junk4c")
                qn2p2 = sbSm.tile([128, 1], F32, name="qn2p2")
                nc.vector.scalar_tensor_tensor(
                    out=junk4c[:], in0=qcol[:].bitcast(F32), scalar=1.0,
                    in1=qcol[:].bitcast(F32), op0=OP.mult, op1=OP.mult,
                    accum_out=qn2p2[:])
                qn2r = sbSm.tile([128, 1], F32, name="qn2r")
                allred(qn2r, qn2p2)

                sT, sq = sT1, sq1

            # ---- final distribution ----
            erow, esum = cos_softmax_rows(sT, sq, qcol, qn2r, F32)
            recS = sbSm.tile([1, 1], F32, name="recS")
            nc.vector.reciprocal(recS[:], esum[:])
            drow = sbRow.tile([1, S], F32, name="drow")
            nc.vector.tensor_scalar(
                out=drow[:], in0=erow[:].bitcast(F32),
                scalar1=recS[0:1, 0:1], scalar2=None, op0=OP.mult)
            nc.sync.dma_start(d_out[b:b + 1, :], drow[:])

            # ---- q to row form ----
            qrowp = psQ.tile([1, D], F32, name="qrowp")
            for t in range(NT):
                nc.tensor.transpose(qrowp[0:1, t * 128:(t + 1) * 128],
                                    qcol[:, t:t + 1].bitcast(F32), eye[:])
            qrow = sbRow.tile([1, D], F32, name="qrow")
            nc.vector.tensor_copy(qrow[:], qrowp[0:1, :])
            nc.sync.dma_start(q_out[b:b + 1, :], qrow[:])

    nc.compile()
    return nc


def kernel(query_emb, state_emb, W, b, ln_gamma, ln_beta):
    key = "nc"
    if key not in _cached:
        _cached[key] = _build()
    nc = _cached[key]

    query_emb = np.asarray(query_emb, dtype=np.float32)
    state_emb = np.asarray(state_emb, dtype=np.float32)
    W = np.asarray(W, dtype=np.float32)
    b = np.asarray(b, dtype=np.float32)
    ln_gamma = np.asarray(ln_gamma, dtype=np.float32)
    ln_beta = np.asarray(ln_beta, dtype=np.float32)

    # host-side layout prep (pure data movement + f32r rounding)
    state_T = _round_f32r(np.ascontiguousarray(state_emb.transpose(0, 2, 1)))
    q_cols = _round_f32r(np.ascontiguousarray(
        query_emb.reshape(B, NT, 128).transpose(0, 2, 1)))
    Wt = _round_f32r(np.ascontiguousarray(W.transpose(0, 2, 1)))
    b_cols = np.ascontiguousarray(b.reshape(N_HOP, NT, 128).transpose(0, 2, 1))
    g_col = np.ascontiguousarray(ln_gamma.reshape(NT, 128).T)
    be_col = np.ascontiguousarray(ln_beta.reshape(NT, 128).T)

    shared = {
        "Wt": Wt, "b_cols": b_cols, "g_col": g_col, "be_col": be_col,
        "ones_c": np.ones((128, 1), np.float32),
        "ones_r": np.ones((1, 128), np.float32),
        "eye_in": np.eye(128, dtype=np.float32),
    }
    in_maps = []
    for c in range(N_CORES):
        sl = slice(c * BL, (c + 1) * BL)
        in_maps.append({"state_T": state_T[sl], "q_cols": q_cols[sl], **shared})

    global _last_in_maps
    _last_in_maps = in_maps
    res = run_bass_kernel_spmd(nc, in_maps, core_ids=list(range(N_CORES)))

    q_full = np.concatenate(
        [res.results[c]["q_out"] for c in range(N_CORES)], axis=0)
    d_full = np.concatenate(
        [res.results[c]["d_out"] for c in range(N_CORES)], axis=0)
    return (q_full.astype(np.float32), d_full.astype(np.float32))


# revision 7
# speedup vs baseline: 1.0032x; 1.0032x over previous
"""Trainium2 Bass kernel for nn_DSI_beta (3-hop DSI pooling).

Reference computation (per batch b):
    s = state_emb[b]           # [S=512, D=512]
    q = query_emb[b]           # [D=512]
    for i in 0..2:
        dist = softmax(10 * cos(q, s_rows))            # [S]
        s = leaky_relu(s @ W[i].T + b[i], 0.01)        # [S, D]
        q = LN(q + dist @ s)                           # [D]
    final_dist = softmax(10 * cos(q, s_rows))
    returns (q, final_dist)

Sharding: data-parallel over batch, 32 batches per NeuronCore (8 cores).

On-chip layout (per batch):
  s is kept TRANSPOSED: sT[t] = s.T[t*128:(t+1)*128, :]  -> 4 tiles [128d, 512S].
  The hop matmul out[e, j] = sum_d W'[d,e] * sT[d,j] keeps the layout invariant
  (weight-stationary on TensorE, fp32r = TF32-like precision, fp32 accumulate).
  q lives as a column tile [128, 4] (q[t*128+p] = qcol[p, t]).
  dot/sumsq rows come from TensorE matvecs; softmax runs on [1,512] rows;
  the weighted aggregation uses a broadcast of exp(logits) over partitions
  (rank-1 matmul) followed by fused multiply+free-reduce on VectorE.
"""

import sys

if "/opt/trn_rl_repo" not in sys.path:
    sys.path.insert(0, "/opt/trn_rl_repo")

from contextlib import ExitStack

import numpy as np

import concourse.tile as tile
from concourse import bacc, bass_isa, mybir
from concourse.bass_utils import run_bass_kernel_spmd

F32 = mybir.dt.float32
F32R = mybir.dt.float32r
AX = mybir.AxisListType
OP = mybir.AluOpType
AF = mybir.ActivationFunctionType

N_CORES = 8
B, S, D = 256, 512, 512
BL = B // N_CORES  # 32 batches per core
NT = D // 128      # 4 partition tiles
N_HOP = 3
TEMP = 10.0
SLOPE = 0.01
EPS_LN = 1e-5

_cached = {}


def _round_f32r(x: np.ndarray) -> np.ndarray:
    """Round fp32 to the 11-bit-mantissa fp32r format (bytes stay fp32)."""
    u = np.ascontiguousarray(x, dtype=np.float32).view(np.uint32)
    u = (u + np.uint32(0x800)) & np.uint32(0xFFFFF000)
    return u.view(np.float32)


def _build(rep=1):
    nc = bacc.Bacc("TRN2", target_bir_lowering=False, debug=False,
                   num_devices=N_CORES)

    state_T = nc.dram_tensor("state_T", [BL, D, S], F32R, kind="ExternalInput").ap()
    q_cols = nc.dram_tensor("q_cols", [BL, 128, NT], F32R, kind="ExternalInput").ap()
    Wt = nc.dram_tensor("Wt", [N_HOP, D, D], F32R, kind="ExternalInput").ap()
    b_cols = nc.dram_tensor("b_cols", [N_HOP, 128, NT], F32, kind="ExternalInput").ap()
    g_col = nc.dram_tensor("g_col", [128, NT], F32, kind="ExternalInput").ap()
    be_col = nc.dram_tensor("be_col", [128, NT], F32, kind="ExternalInput").ap()
    ones_c = nc.dram_tensor("ones_c", [128, 1], F32R, kind="ExternalInput").ap()
    ones_r = nc.dram_tensor("ones_r", [1, 128], F32R, kind="ExternalInput").ap()
    eye_in = nc.dram_tensor("eye_in", [128, 128], F32, kind="ExternalInput").ap()

    q_out = nc.dram_tensor("q_out", [BL, D], F32, kind="ExternalOutput").ap()
    d_out = nc.dram_tensor("d_out", [BL, S], F32, kind="ExternalOutput").ap()

    with tile.TileContext(nc) as tc, ExitStack() as ctx:
        const = ctx.enter_context(tc.tile_pool(name="const", bufs=1))
        sbST = ctx.enter_context(tc.tile_pool(name="sbST", bufs=12))
        sbSQ = ctx.enter_context(tc.tile_pool(name="sbSQ", bufs=12))
        sbQ = ctx.enter_context(tc.tile_pool(name="sbQ", bufs=4))
        sbRow = ctx.enter_context(tc.tile_pool(name="sbRow", bufs=4))
        sbSm = ctx.enter_context(tc.tile_pool(name="sbSm", bufs=6))
        sbJk = ctx.enter_context(tc.tile_pool(name="sbJk", bufs=3))
        psMM = ctx.enter_context(tc.tile_pool(name="psMM", bufs=2, space="PSUM"))
        psDot = ctx.enter_context(tc.tile_pool(name="psDot", bufs=1, space="PSUM"))
        psSS = ctx.enter_context(tc.tile_pool(name="psSS", bufs=1, space="PSUM"))
        psE = ctx.enter_context(tc.tile_pool(name="psE", bufs=1, space="PSUM"))
        psQ = ctx.enter_context(tc.tile_pool(name="psQ", bufs=1, space="PSUM"))

        # ---- constants (loaded once) ----
        Wtiles = []  # Wtiles[i][t]: [128, 512] = Wt[i, t*128:(t+1)*128, :]
        for i in range(N_HOP):
            row = []
            for t in range(NT):
                w = const.tile([128, D], F32R, name=f"W_{i}_{t}")
                nc.sync.dma_start(w[:], Wt[i, t * 128:(t + 1) * 128, :])
                row.append(w)
            Wtiles.append(row)
        bcols = []
        for i in range(N_HOP):
            bc = const.tile([128, NT], F32, name=f"b_{i}")
            nc.sync.dma_start(bc[:], b_cols[i])
            bcols.append(bc)
        gcol = const.tile([128, NT], F32, name="gcol")
        nc.sync.dma_start(gcol[:], g_col)
        becol = const.tile([128, NT], F32, name="becol")
        nc.sync.dma_start(becol[:], be_col)
        onesc = const.tile([128, 1], F32R, name="onesc")
        nc.sync.dma_start(onesc[:], ones_c)
        onesr = const.tile([1, 128], F32R, name="onesr")
        nc.sync.dma_start(onesr[:], ones_r)
        eye = const.tile([128, 128], F32, name="eye")
        nc.sync.dma_start(eye[:], eye_in)
        epsln = const.tile([128, 1], F32, name="epsln")
        nc.vector.memset(epsln[:], EPS_LN)

        def allred(dst, src):
            nc.gpsimd.partition_all_reduce(dst[:], src[:], channels=128,
                                           reduce_op=bass_isa.ReduceOp.add)

        for _rep in range(rep):
          for b in range(BL):
            # ---- load state (transposed on host) and query column ----
            sT = []
            for t in range(NT):
                st = sbST.tile([128, S], F32R, name="sT")
                nc.sync.dma_start(st[:], state_T[b, t * 128:(t + 1) * 128, :])
                sT.append(st)
            qcol = sbQ.tile([128, NT], F32R, name="qcol")
            nc.sync.dma_start(qcol[:], q_cols[b])

            # squares of s (for row norms)
            sq = []
            for t in range(NT):
                s2 = sbSQ.tile([128, S], F32R, name="sq")
                nc.scalar.activation(s2[:], sT[t][:].bitcast(F32), AF.Square)
                sq.append(s2)

            # ||q||^2 replicated across partitions
            junk4 = sbSm.tile([128, NT], F32, name="junk4")
            qn2p = sbSm.tile([128, 1], F32, name="qn2p")
            nc.vector.scalar_tensor_tensor(
                out=junk4[:], in0=qcol[:].bitcast(F32), scalar=1.0,
                in1=qcol[:].bitcast(F32), op0=OP.mult, op1=OP.mult,
                accum_out=qn2p[:])
            qn2r = sbSm.tile([128, 1], F32, name="qn2r")
            allred(qn2r, qn2p)

            def cos_softmax_rows(sT_i, sq_i, qcol_i, qn2r_i, erow_dtype):
                """dot/sumsq matvecs + row softmax numerator.

                Returns (erow [1,512] tile of erow_dtype, esum [1,1] f32).
                erow = exp(10 * cos), esum = sum(erow).
                """
                dotp = psDot.tile([1, S], F32, name="dotp")
                for t in range(NT):
                    nc.tensor.matmul(dotp[0:1, :], qcol_i[:, t:t + 1], sT_i[t][:],
                                     start=(t == 0), stop=(t == NT - 1))
                ssp = psSS.tile([1, S], F32, name="ssp")
                for t in range(NT):
                    nc.tensor.matmul(ssp[0:1, :], onesc[:], sq_i[t][:],
                                     start=(t == 0), stop=(t == NT - 1))
                # rec = 1/(sn*qn) = exp(-0.5*ln(sumsq * qn^2))
                lnx = sbRow.tile([1, S], F32, name="lnx")
                nc.scalar.activation(lnx[:], ssp[0:1, :], AF.Ln,
                                     scale=qn2r_i[0:1, 0:1])
                rec = sbRow.tile([1, S], F32, name="rec")
                nc.scalar.activation(rec[:], lnx[:], AF.Exp, scale=-0.5)
                lg = sbRow.tile([1, S], F32, name="lg")
                nc.vector.tensor_mul(lg[:], dotp[0:1, :], rec[:])
                erow = sbRow.tile([1, S], erow_dtype, name="erow")
                esum = sbSm.tile([1, 1], F32, name="esum")
                nc.scalar.activation(erow[:], lg[:], AF.Exp, scale=TEMP,
                                     accum_out=esum[:])
                return erow, esum

            for i in range(N_HOP):
                erow, esum = cos_softmax_rows(sT, sq, qcol, qn2r, F32R)

                # ---- big matmul: s_{i+1}^T[e, j] = lrelu(sum_d W'[d,e] sT[d,j] + b[e])
                sT1, sq1 = [], []
                for m in range(NT):
                    mmp = psMM.tile([128, S], F32, name="mmp")
                    for t in range(NT):
                        nc.tensor.matmul(
                            mmp[:], Wtiles[i][t][:, m * 128:(m + 1) * 128],
                            sT[t][:], start=(t == 0), stop=(t == NT - 1))
                    st1 = sbST.tile([128, S], F32R, name="sT")
                    nc.scalar.activation(st1[:], mmp[:], AF.Prelu,
                                         bias=bcols[i][:, m:m + 1], scale=1.0,
                                         alpha=SLOPE)
                    sT1.append(st1)
                    s21 = sbSQ.tile([128, S], F32R, name="sq")
                    nc.scalar.activation(s21[:], st1[:].bitcast(F32), AF.Square)
                    sq1.append(s21)

                # ---- agg[e] = sum_j erow[j] * s_{i+1}^T[e, j] ----
                Ep = psE.tile([128, S], F32, name="Ep")
                nc.tensor.matmul(Ep[:], onesr[:], erow[:], start=True, stop=True)
                aggc = sbSm.tile([128, NT], F32, name="aggc")
                for t in range(NT):
                    jk = sbJk.tile([128, S], F32, name="jk")
                    nc.vector.scalar_tensor_tensor(
                        out=jk[:], in0=sT1[t][:].bitcast(F32), scalar=1.0,
                        in1=Ep[:], op0=OP.mult, op1=OP.mult,
                        accum_out=aggc[:, t:t + 1])

                # ---- q = LN(q + agg/esum) in column land ----
                recS = sbSm.tile([1, 1], F32, name="recS")
                nc.vector.reciprocal(recS[:], esum[:])
                recSb = sbSm.tile([128, 1], F32, name="recSb")
                nc.gpsimd.partition_broadcast(recSb[:], recS[:], channels=128)
                qpre = sbSm.tile([128, NT], F32, name="qpre")
                qsum = sbSm.tile([128, 1], F32, name="qsum")
                nc.vector.scalar_tensor_tensor(
                    out=qpre[:], in0=aggc[:], scalar=recSb[:],
                    in1=qcol[:].bitcast(F32), op0=OP.mult, op1=OP.add,
                    accum_out=qsum[:])
                qsumr = sbSm.tile([128, 1], F32, name="qsumr")
                allred(qsumr, qsum)
                negmu = sbSm.tile([128, 1], F32, name="negmu")
                nc.vector.tensor_scalar(out=negmu[:], in0=qsumr[:],
                                        scalar1=-1.0 / D, scalar2=None,
                                        op0=OP.mult)
                center = sbSm.tile([128, NT], F32, name="center")
                nc.vector.tensor_scalar(out=center[:], in0=qpre[:],
                                        scalar1=negmu[:], scalar2=None,
                                        op0=OP.add)
                junk4b = sbSm.tile([128, NT], F32, name="junk4b")
                vs = sbSm.tile([128, 1], F32, name="vs")
                nc.vector.scalar_tensor_tensor(
                    out=junk4b[:], in0=center[:], scalar=1.0, in1=center[:],
                    op0=OP.mult, op1=OP.mult, accum_out=vs[:])
                vsr = sbSm.tile([128, 1], F32, name="vsr")
                allred(vsr, vs)
                lnv = sbSm.tile([128, 1], F32, name="lnv")
                nc.scalar.activation(lnv[:], vsr[:], AF.Ln, scale=1.0 / D,
                                     bias=epsln[:, 0:1])
                rstd = sbSm.tile([128, 1], F32, name="rstd")
                nc.scalar.activation(rstd[:], lnv[:], AF.Exp, scale=-0.5)
                qtmp = sbSm.tile([128, NT], F32, name="qtmp")
                nc.vector.scalar_tensor_tensor(
                    out=qtmp[:], in0=center[:], scalar=rstd[:], in1=gcol[:],
                    op0=OP.mult, op1=OP.mult)
                qcol = sbQ.tile([128, NT], F32R, name="qcol")
                nc.vector.tensor_tensor(out=qcol[:], in0=qtmp[:], in1=becol[:],
                                        op=OP.add)
                # ||q||^2 for the next cosine
                junk4c = sbSm.tile([128, NT], F32, name="junk4c")
                qn2p2 = sbSm.tile([128, 1], F32, name="qn2p2")
                nc.vector.scalar_tensor_tensor(
                    out=junk4c[:], in0=qcol[:].bitcast(F32), scalar=1.0,
                    in1=qcol[:].bitcast(F32), op0=OP.mult, op1=OP.mult,
                    accum_out=qn2p2[:])
                qn2r = sbSm.tile([128, 1], F32, name="qn2r")
                allred(qn2r, qn2p2)

                sT, sq = sT1, sq1

            # ---- final distribution ----
            erow, esum = cos_softmax_rows(sT, sq, qcol, qn2r, F32)
            recS = sbSm.tile([1, 1], F32, name="recS")
            nc.vector.reciprocal(recS[:], esum[:])
            drow = sbRow.tile([1, S], F32, name="drow")
            nc.vector.tensor_scalar(
                out=drow[:], in0=erow[:].bitcast(F32),
                scalar1=recS[0:1, 0:1], scalar2=None, op0=OP.mult)
            nc.sync.dma_start(d_out[b:b + 1, :], drow[:])

            # ---- q to row form ----
            qrowp = psQ.tile([1, D], F32, name="qrowp")
            for t in range(NT):
                nc.tensor.transpose(qrowp[0:1, t * 128:(t + 1) * 128],
                                    qcol[:, t:t + 1].bitcast(F32), eye[:])
            qrow = sbRow.tile([1, D], F32, name="qrow")
            nc.vector.tensor_copy(qrow[:], qrowp[0:1, :])
            nc.sync.dma_start(q_out[b:b + 1, :], qrow[:])

    nc.compile()
    return nc


def kernel(query_emb, state_emb, W, b, ln_gamma, ln_beta):
    key = "nc"
    if key not in _cached:
        _cached[key] = _build()
    nc = _cached[key]

    query_emb = np.asarray(query_emb, dtype=np.float32)
    state_emb = np.asarray(state_emb, dtype=np.float32)
    W = np.asarray(W, dtype=np.float32)
    b = np.asarray(b, dtype=np.float32)
    ln_gamma = np.asarray(ln_gamma, dtype=np.float32)
    ln_beta = np.asarray(ln_beta, dtype=np.float32)

    # host-side layout prep (pure data movement + f32r rounding)
    state_T = _round_f32r(np.ascontiguousarray(state_emb.transpose(0, 2, 1)))
    q_cols = _round_f32r(np.ascontiguousarray(
        query_emb.reshape(B, NT, 128).transpose(0, 2, 1)))
    Wt = _round_f32r(np.ascontiguousarray(W.transpose(0, 2, 1)))
    b_cols = np.ascontiguousarray(b.reshape(N_HOP, NT, 128).transpose(0, 2, 1))
    g_col = np.ascontiguousarray(ln_gamma.reshape(NT, 128).T)
    be_col = np.ascontiguousarray(ln_beta.reshape(NT, 128).T)

    shared = {
        "Wt": Wt, "b_cols": b_cols, "g_col": g_col, "be_col": be_col,
        "ones_c": np.ones((128, 1), np.float32),
        "ones_r": np.ones((1, 128), np.float32),
        "eye_in": np.eye(128, dtype=np.float32),
    }
    in_maps = []
    for c in range(N_CORES):
        sl = slice(c * BL, (c + 1) * BL)
        in_maps.append({"state_T": state_T[sl], "q_cols": q_cols[sl], **shared})

    global _last_in_maps
    _last_in_maps = in_maps
    res = run_bass_kernel_spmd(nc, in_maps, core_ids=list(range(N_CORES)))

    q_full = np.concatenate(
        [res.results[c]["q_out"] for c in range(N_CORES)], axis=0)
    d_full = np.concatenate(
        [res.results[c]["d_out"] for c in range(N_CORES)], axis=0)
    return (q_full.astype(np.float32), d_full.astype(np.float32))


# revision 8
# speedup vs baseline: 1.4175x; 1.4130x over previous
"""Trainium2 Bass kernel for nn_DSI_beta (3-hop DSI pooling).

Reference computation (per batch b):
    s = state_emb[b]           # [S=512, D=512]
    q = query_emb[b]           # [D=512]
    for i in 0..2:
        dist = softmax(10 * cos(q, s_rows))            # [S]
        s = leaky_relu(s @ W[i].T + b[i], 0.01)        # [S, D]
        q = LN(q + dist @ s)                           # [D]
    final_dist = softmax(10 * cos(q, s_rows))
    returns (q, final_dist)

Sharding: data-parallel over batch, 32 batches per NeuronCore (8 cores).

On-chip layout (per batch):
  s is kept TRANSPOSED: sT[t] = s.T[t*128:(t+1)*128, :]  -> 4 tiles [128d, 512S].
  The hop matmul out[e, j] = sum_d W'[d,e] * sT[d,j] keeps the layout invariant
  (weight-stationary on TensorE, fp32r = TF32-like precision, fp32 accumulate).
  q lives as a column tile [128, 4] (q[t*128+p] = qcol[p, t]).
  dot/sumsq rows come from TensorE matvecs; softmax runs on [1,512] rows;
  the weighted aggregation uses a broadcast of exp(logits) over partitions
  (rank-1 matmul) followed by fused multiply+free-reduce on VectorE.
"""

import sys

if "/opt/trn_rl_repo" not in sys.path:
    sys.path.insert(0, "/opt/trn_rl_repo")

from contextlib import ExitStack

import numpy as np

import concourse.tile as tile
from concourse import bacc, bass_isa, mybir
from concourse.bass_utils import run_bass_kernel_spmd

# All activations in this kernel (Square, Prelu, Ln, Exp) live in the
# natural_log_exp_and_others table set. The stock table-choice pass greedily
# picks the first set containing each function (exp_and_others for Exp,
# natural_log for Ln), inserting ~450 table loads (~1.3us each). Restrict the
# visible tables to the one set that covers everything so exactly one load is
# emitted. Index positions must be preserved (act_func_set_id = list index).
_ONE_SET = "natural_log_exp_and_others"
_orig_get_tables = None


def _patched_get_tables(arch):
    full = _orig_get_tables(arch)
    return {name: (funcs if name == _ONE_SET else set())
            for name, funcs in full.items()}


def _install_table_patch():
    global _orig_get_tables
    if _orig_get_tables is None:
        _orig_get_tables = bacc.get_activation_tables
        bacc.get_activation_tables = _patched_get_tables

F32 = mybir.dt.float32
F32R = mybir.dt.float32r
AX = mybir.AxisListType
OP = mybir.AluOpType
AF = mybir.ActivationFunctionType

N_CORES = 8
B, S, D = 256, 512, 512
BL = B // N_CORES  # 32 batches per core
NT = D // 128      # 4 partition tiles
N_HOP = 3
TEMP = 10.0
SLOPE = 0.01
EPS_LN = 1e-5

_cached = {}


def _round_f32r(x: np.ndarray) -> np.ndarray:
    """Round fp32 to the 11-bit-mantissa fp32r format (bytes stay fp32)."""
    u = np.ascontiguousarray(x, dtype=np.float32).view(np.uint32)
    u = (u + np.uint32(0x800)) & np.uint32(0xFFFFF000)
    return u.view(np.float32)


def _build(rep=1):
    _install_table_patch()
    nc = bacc.Bacc("TRN2", target_bir_lowering=False, debug=False,
                   num_devices=N_CORES)

    state_T = nc.dram_tensor("state_T", [BL, D, S], F32R, kind="ExternalInput").ap()
    q_cols = nc.dram_tensor("q_cols", [BL, 128, NT], F32R, kind="ExternalInput").ap()
    Wt = nc.dram_tensor("Wt", [N_HOP, D, D], F32R, kind="ExternalInput").ap()
    b_cols = nc.dram_tensor("b_cols", [N_HOP, 128, NT], F32, kind="ExternalInput").ap()
    g_col = nc.dram_tensor("g_col", [128, NT], F32, kind="ExternalInput").ap()
    be_col = nc.dram_tensor("be_col", [128, NT], F32, kind="ExternalInput").ap()
    ones_c = nc.dram_tensor("ones_c", [128, 1], F32R, kind="ExternalInput").ap()
    ones_r = nc.dram_tensor("ones_r", [1, 128], F32R, kind="ExternalInput").ap()
    eye_in = nc.dram_tensor("eye_in", [128, 128], F32, kind="ExternalInput").ap()

    q_out = nc.dram_tensor("q_out", [BL, D], F32, kind="ExternalOutput").ap()
    d_out = nc.dram_tensor("d_out", [BL, S], F32, kind="ExternalOutput").ap()

    with tile.TileContext(nc) as tc, ExitStack() as ctx:
        const = ctx.enter_context(tc.tile_pool(name="const", bufs=1))
        sbST = ctx.enter_context(tc.tile_pool(name="sbST", bufs=12))
        sbSQ = ctx.enter_context(tc.tile_pool(name="sbSQ", bufs=12))
        sbQ = ctx.enter_context(tc.tile_pool(name="sbQ", bufs=4))
        sbRow = ctx.enter_context(tc.tile_pool(name="sbRow", bufs=4))
        sbSm = ctx.enter_context(tc.tile_pool(name="sbSm", bufs=6))
        sbJk = ctx.enter_context(tc.tile_pool(name="sbJk", bufs=3))
        psMM = ctx.enter_context(tc.tile_pool(name="psMM", bufs=2, space="PSUM"))
        psDot = ctx.enter_context(tc.tile_pool(name="psDot", bufs=1, space="PSUM"))
        psSS = ctx.enter_context(tc.tile_pool(name="psSS", bufs=1, space="PSUM"))
        psE = ctx.enter_context(tc.tile_pool(name="psE", bufs=1, space="PSUM"))
        psQ = ctx.enter_context(tc.tile_pool(name="psQ", bufs=1, space="PSUM"))

        # ---- constants (loaded once) ----
        Wtiles = []  # Wtiles[i][t]: [128, 512] = Wt[i, t*128:(t+1)*128, :]
        for i in range(N_HOP):
            row = []
            for t in range(NT):
                w = const.tile([128, D], F32R, name=f"W_{i}_{t}")
                nc.sync.dma_start(w[:], Wt[i, t * 128:(t + 1) * 128, :])
                row.append(w)
            Wtiles.append(row)
        bcols = []
        for i in range(N_HOP):
            bc = const.tile([128, NT], F32, name=f"b_{i}")
            nc.sync.dma_start(bc[:], b_cols[i])
            bcols.append(bc)
        gcol = const.tile([128, NT], F32, name="gcol")
        nc.sync.dma_start(gcol[:], g_col)
        becol = const.tile([128, NT], F32, name="becol")
        nc.sync.dma_start(becol[:], be_col)
        onesc = const.tile([128, 1], F32R, name="onesc")
        nc.sync.dma_start(onesc[:], ones_c)
        onesr = const.tile([1, 128], F32R, name="onesr")
        nc.sync.dma_start(onesr[:], ones_r)
        eye = const.tile([128, 128], F32, name="eye")
        nc.sync.dma_start(eye[:], eye_in)
        epsln = const.tile([128, 1], F32, name="epsln")
        nc.vector.memset(epsln[:], EPS_LN)

        def allred(dst, src):
            nc.gpsimd.partition_all_reduce(dst[:], src[:], channels=128,
                                           reduce_op=bass_isa.ReduceOp.add)

        for _rep in range(rep):
          for b in range(BL):
            # ---- load state (transposed on host) and query column ----
            sT = []
            for t in range(NT):
                st = sbST.tile([128, S], F32R, name="sT")
                nc.sync.dma_start(st[:], state_T[b, t * 128:(t + 1) * 128, :])
                sT.append(st)
            qcol = sbQ.tile([128, NT], F32R, name="qcol")
            nc.sync.dma_start(qcol[:], q_cols[b])

            # squares of s (for row norms)
            sq = []
            for t in range(NT):
                s2 = sbSQ.tile([128, S], F32R, name="sq")
                nc.scalar.activation(s2[:], sT[t][:].bitcast(F32), AF.Square)
                sq.append(s2)

            # ||q||^2 replicated across partitions
            junk4 = sbSm.tile([128, NT], F32, name="junk4")
            qn2p = sbSm.tile([128, 1], F32, name="qn2p")
            nc.vector.scalar_tensor_tensor(
                out=junk4[:], in0=qcol[:].bitcast(F32), scalar=1.0,
                in1=qcol[:].bitcast(F32), op0=OP.mult, op1=OP.mult,
                accum_out=qn2p[:])
            qn2r = sbSm.tile([128, 1], F32, name="qn2r")
            allred(qn2r, qn2p)

            def cos_softmax_rows(sT_i, sq_i, qcol_i, qn2r_i, erow_dtype):
                """dot/sumsq matvecs + row softmax numerator.

                Returns (erow [1,512] tile of erow_dtype, esum [1,1] f32).
                erow = exp(10 * cos), esum = sum(erow).
                """
                dotp = psDot.tile([1, S], F32, name="dotp")
                for t in range(NT):
                    nc.tensor.matmul(dotp[0:1, :], qcol_i[:, t:t + 1], sT_i[t][:],
                                     start=(t == 0), stop=(t == NT - 1))
                ssp = psSS.tile([1, S], F32, name="ssp")
                for t in range(NT):
                    nc.tensor.matmul(ssp[0:1, :], onesc[:], sq_i[t][:],
                                     start=(t == 0), stop=(t == NT - 1))
                # rec = 1/(sn*qn) = exp(-0.5*ln(sumsq * qn^2))
                lnx = sbRow.tile([1, S], F32, name="lnx")
                nc.scalar.activation(lnx[:], ssp[0:1, :], AF.Ln,
                                     scale=qn2r_i[0:1, 0:1])
                rec = sbRow.tile([1, S], F32, name="rec")
                nc.scalar.activation(rec[:], lnx[:], AF.Exp, scale=-0.5)
                lg = sbRow.tile([1, S], F32, name="lg")
                nc.vector.tensor_mul(lg[:], dotp[0:1, :], rec[:])
                erow = sbRow.tile([1, S], erow_dtype, name="erow")
                esum = sbSm.tile([1, 1], F32, name="esum")
                nc.scalar.activation(erow[:], lg[:], AF.Exp, scale=TEMP,
                                     accum_out=esum[:])
                return erow, esum

            for i in range(N_HOP):
                erow, esum = cos_softmax_rows(sT, sq, qcol, qn2r, F32R)

                # ---- big matmul: s_{i+1}^T[e, j] = lrelu(sum_d W'[d,e] sT[d,j] + b[e])
                sT1, sq1 = [], []
                for m in range(NT):
                    mmp = psMM.tile([128, S], F32, name="mmp")
                    for t in range(NT):
                        nc.tensor.matmul(
                            mmp[:], Wtiles[i][t][:, m * 128:(m + 1) * 128],
                            sT[t][:], start=(t == 0), stop=(t == NT - 1))
                    st1 = sbST.tile([128, S], F32R, name="sT")
                    nc.scalar.activation(st1[:], mmp[:], AF.Prelu,
                                         bias=bcols[i][:, m:m + 1], scale=1.0,
                                         alpha=SLOPE)
                    sT1.append(st1)
                    s21 = sbSQ.tile([128, S], F32R, name="sq")
                    nc.scalar.activation(s21[:], st1[:].bitcast(F32), AF.Square)
                    sq1.append(s21)

                # ---- agg[e] = sum_j erow[j] * s_{i+1}^T[e, j] ----
                Ep = psE.tile([128, S], F32, name="Ep")
                nc.tensor.matmul(Ep[:], onesr[:], erow[:], start=True, stop=True)
                aggc = sbSm.tile([128, NT], F32, name="aggc")
                for t in range(NT):
                    jk = sbJk.tile([128, S], F32, name="jk")
                    nc.vector.scalar_tensor_tensor(
                        out=jk[:], in0=sT1[t][:].bitcast(F32), scalar=1.0,
                        in1=Ep[:], op0=OP.mult, op1=OP.mult,
                        accum_out=aggc[:, t:t + 1])

                # ---- q = LN(q + agg/esum) in column land ----
                recS = sbSm.tile([1, 1], F32, name="recS")
                nc.vector.reciprocal(recS[:], esum[:])
                recSb = sbSm.tile([128, 1], F32, name="recSb")
                nc.gpsimd.partition_broadcast(recSb[:], recS[:], channels=128)
                qpre = sbSm.tile([128, NT], F32, name="qpre")
                qsum = sbSm.tile([128, 1], F32, name="qsum")
                nc.vector.scalar_tensor_tensor(
                    out=qpre[:], in0=aggc[:], scalar=recSb[:],
                    in1=qcol[:].bitcast(F32), op0=OP.mult, op1=OP.add,
                    accum_out=qsum[:])
                qsumr = sbSm.tile([128, 1], F32, name="qsumr")
                allred(qsumr, qsum)
                negmu = sbSm.tile([128, 1], F32, name="negmu")
                nc.vector.tensor_scalar(out=negmu[:], in0=qsumr[:],
                                        scalar1=-1.0 / D, scalar2=None,
                                        op0=OP.mult)
                center = sbSm.tile([128, NT], F32, name="center")
                nc.vector.tensor_scalar(out=center[:], in0=qpre[:],
                                        scalar1=negmu[:], scalar2=None,
                                        op0=OP.add)
                junk4b = sbSm.tile([128, NT], F32, name="junk4b")
                vs = sbSm.tile([128, 1], F32, name="vs")
                nc.vector.scalar_tensor_tensor(
                    out=junk4b[:], in0=center[:], scalar=1.0, in1=center[:],
                    op0=OP.mult, op1=OP.mult, accum_out=vs[:])
                vsr = sbSm.tile([128, 1], F32, name="vsr")
                allred(vsr, vs)
                lnv = sbSm.tile([128, 1], F32, name="lnv")
                nc.scalar.activation(lnv[:], vsr[:], AF.Ln, scale=1.0 / D,
                                     bias=epsln[:, 0:1])
                rstd = sbSm.tile([128, 1], F32, name="rstd")
                nc.scalar.activation(rstd[:], lnv[:], AF.Exp, scale=-0.5)
                qtmp = sbSm.tile([128, NT], F32, name="qtmp")
                nc.vector.scalar_tensor_tensor(
                    out=qtmp[:], in0=center[:], scalar=rstd[:], in1=gcol[:],
                    op0=OP.mult, op1=OP.mult)
                qcol = sbQ.tile([128, NT], F32R, name="qcol")
                nc.vector.tensor_tensor(out=qcol[:], in0=qtmp[:], in1=becol[:],
                                        op=OP.add)
                # ||q||^2 for the next cosine
                junk4c = sbSm.tile([128, NT], F32, name="junk4c")
                qn2p2 = sbSm.tile([128, 1], F32, name="qn2p2")
                nc.vector.scalar_tensor_tensor(
                    out=junk4c[:], in0=qcol[:].bitcast(F32), scalar=1.0,
                    in1=qcol[:].bitcast(F32), op0=OP.mult, op1=OP.mult,
                    accum_out=qn2p2[:])
                qn2r = sbSm.tile([128, 1], F32, name="qn2r")
                allred(qn2r, qn2p2)

                sT, sq = sT1, sq1

            # ---- final distribution ----
            erow, esum = cos_softmax_rows(sT, sq, qcol, qn2r, F32)
            recS = sbSm.tile([1, 1], F32, name="recS")
            nc.vector.reciprocal(recS[:], esum[:])
            drow = sbRow.tile([1, S], F32, name="drow")
            nc.vector.tensor_scalar(
                out=drow[:], in0=erow[:].bitcast(F32),
                scalar1=recS[0:1, 0:1], scalar2=None, op0=OP.mult)
            nc.sync.dma_start(d_out[b:b + 1, :], drow[:])

            # ---- q to row form ----
            qrowp = psQ.tile([1, D], F32, name="qrowp")
            for t in range(NT):
                nc.tensor.transpose(qrowp[0:1, t * 128:(t + 1) * 128],
                                    qcol[:, t:t + 1].bitcast(F32), eye[:])
            qrow = sbRow.tile([1, D], F32, name="qrow")
            nc.vector.tensor_copy(qrow[:], qrowp[0:1, :])
            nc.sync.dma_start(q_out[b:b + 1, :], qrow[:])

    nc.compile()
    return nc


def kernel(query_emb, state_emb, W, b, ln_gamma, ln_beta):
    key = "nc"
    if key not in _cached:
        _cached[key] = _build()
    nc = _cached[key]

    query_emb = np.asarray(query_emb, dtype=np.float32)
    state_emb = np.asarray(state_emb, dtype=np.float32)
    W = np.asarray(W, dtype=np.float32)
    b = np.asarray(b, dtype=np.float32)
    ln_gamma = np.asarray(ln_gamma, dtype=np.float32)
    ln_beta = np.asarray(ln_beta, dtype=np.float32)

    # host-side layout prep (pure data movement + f32r rounding)
    state_T = _round_f32r(np.ascontiguousarray(state_emb.transpose(0, 2, 1)))
    q_cols = _round_f32r(np.ascontiguousarray(
        query_emb.reshape(B, NT, 128).transpose(0, 2, 1)))
    Wt = _round_f32r(np.ascontiguousarray(W.transpose(0, 2, 1)))
    b_cols = np.ascontiguousarray(b.reshape(N_HOP, NT, 128).transpose(0, 2, 1))
    g_col = np.ascontiguousarray(ln_gamma.reshape(NT, 128).T)
    be_col = np.ascontiguousarray(ln_beta.reshape(NT, 128).T)

    shared = {
        "Wt": Wt, "b_cols": b_cols, "g_col": g_col, "be_col": be_col,
        "ones_c": np.ones((128, 1), np.float32),
        "ones_r": np.ones((1, 128), np.float32),
        "eye_in": np.eye(128, dtype=np.float32),
    }
    in_maps = []
    for c in range(N_CORES):
        sl = slice(c * BL, (c + 1) * BL)
        in_maps.append({"state_T": state_T[sl], "q_cols": q_cols[sl], **shared})

    global _last_in_maps
    _last_in_maps = in_maps
    res = run_bass_kernel_spmd(nc, in_maps, core_ids=list(range(N_CORES)))

    q_full = np.concatenate(
        [res.results[c]["q_out"] for c in range(N_CORES)], axis=0)
    d_full = np.concatenate(
        [res.results[c]["d_out"] for c in range(N_CORES)], axis=0)
    return (q_full.astype(np.float32), d_full.astype(np.float32))
